# revision 27
# baseline (speedup 1.0000x reference)
"""MultiHeadedAttention Trainium2 kernel (8 NeuronCores, SPMD).

Sharding: core c -> batch b = c//4, head-group r = c%4 (4 of 16 heads).
Host compacts BOTH keys and queries by the shared mask (~50% ones) to
npad=1152 slots, casts everything to fp16, and scatters valid-query
columns back; masked-query outputs are the constant vmean row computed
on host. Per core:
    xqT/xkT/xvT/wT     via DMA-transpose loads (fp16, no PE transposes)
    qT = (wq_r @ xq^T) + bq_r            [256, QP]  (dk-major)
    kT = (wk_r @ xk^T) + bk_r            [256, KP]
    v  = (xv @ wv_r^T + bv_r) * vones    [KP, 256]  (+ ones col = denom)
    sT_h = kT_h^T @ qT_h   (scores [key, q]); z = exp(sT/8) fp16
    ctx[q, dk+1] = z_sub^T @ v_aug   accumulated over key tiles
    normalize per-partition (q) by 1/denom; PE-transpose to ctxT [dk, q]
    outT_partial = wo_r^T @ ctxT         [D, QP]
Host: out[b][valid] = sum_r outT^T + bo; masked rows = vmean @ wo^T + bo.

Self-contained: hardcodes B=2, S=2048, D=1024, H=16.
"""

import numpy as np
from contextlib import ExitStack

import concourse.bacc as bacc
import concourse.tile as tile
from concourse import mybir
from concourse.bass_utils import run_bass_kernel_spmd
from concourse.masks import make_identity

F32 = mybir.dt.float32
F16 = mybir.dt.float16
AF = mybir.ActivationFunctionType

B, S, D, H = 2, 2048, 1024, 16
DK = 64                      # head dim
HC = 4                       # heads per core
DH = HC * DK                 # 256, local head width
MB = DH // 128               # 2 head-pair blocks
PD = D // 128                # 8 d-blocks
VW = DK + 1                  # 65: v columns + denominator ones column
NCORES = 8

_cache = {}
NKP = 1152                   # padded compacted slot count (mask ~ Bern 1/2)


def _build_nc(npad=NKP):
    """Per-core Bass program (identical on all 8 cores). npad = padded
    count for both queries and keys (1152 main, 2048 fallback)."""
    QC = 384 if npad % 384 == 0 else 512   # query chunk (psum-bank bound)
    NJ = npad // QC
    NKT = npad // 128        # key tiles
    QSUB = QC // 128         # q subtiles per chunk (3 or 4)
    GW = QSUB * VW           # ctx psum group width per head (195 or 260)

    nc = bacc.Bacc("TRN2", target_bir_lowering=False, debug=False,
                   num_devices=NCORES)

    xq_d = nc.dram_tensor("xq", [npad, D], F16, kind="ExternalInput").ap()
    xk_d = nc.dram_tensor("xk", [npad, D], F16, kind="ExternalInput").ap()
    xv_d = nc.dram_tensor("xv", [npad, D], F16, kind="ExternalInput").ap()
    wq_d = nc.dram_tensor("wq", [DH, D], F16, kind="ExternalInput").ap()
    wk_d = nc.dram_tensor("wk", [DH, D], F16, kind="ExternalInput").ap()
    wv_d = nc.dram_tensor("wv", [DH, D], F16, kind="ExternalInput").ap()
    wo_d = nc.dram_tensor("wo", [D, DH], F16, kind="ExternalInput").ap()
    bv_d = nc.dram_tensor("bv", [DH], F32, kind="ExternalInput").ap()
    # packed consts: cols 0:NKT*HC vones, then bq [MB], bk [MB] (p-major)
    cst_d = nc.dram_tensor("cst", [128, NKT * HC + 2 * MB], F32,
                           kind="ExternalInput").ap()
    outT_d = nc.dram_tensor("outT", [D, npad], F16, kind="ExternalOutput").ap()

    with tile.TileContext(nc) as tc, ExitStack() as top:
        const = top.enter_context(tc.tile_pool(name="const", bufs=1))
        ident = const.tile([128, 128], F16)
        make_identity(nc, ident)
        ones_row = const.tile([1, 128], F16)
        nc.vector.memset(ones_row[:], 1.0)
        cst = const.tile([128, NKT * HC + 2 * MB], F32)
        nc.sync.dma_start(out=cst[:], in_=cst_d[:, :])
        vones = cst[:, 0:NKT * HC]
        bq_sb = cst[:, NKT * HC:NKT * HC + MB]
        bk_sb = cst[:, NKT * HC + MB:NKT * HC + 2 * MB]
        bv_row = const.tile([1, DH], F32)
        nc.sync.dma_start(out=bv_row[:], in_=bv_d[None, :])
        bv_row16 = const.tile([1, DH], F16)
        nc.vector.tensor_copy(bv_row16[:], bv_row[:])
        bv_rep = const.tile([128, DH], F16)

        # weight tiles, transposed on load: [128 d-part, kc-block * width]
        wqt = const.tile([128, PD * DH], F16)
        wkt = const.tile([128, PD * DH], F16)
        wvt = const.tile([128, PD * DH], F16)
        wot = const.tile([128, MB * D], F16)
        # x transposed: [128 d-part, kc-block * npad]
        xqT = const.tile([128, PD * npad], F16)
        xkT = const.tile([128, PD * npad], F16)
        xvT = const.tile([128, PD * npad], F16)

        qT = const.tile([128, MB * npad], F16)    # [dk-pair, m*npad + s]
        kT = const.tile([128, MB * npad], F16)
        v_aug = const.tile([128, NKT * HC * VW], F16)
        ctx_sb = const.tile([128, MB * npad], F16)

        vag = v_aug[:].rearrange("p (t h c) -> p t h c", t=NKT, h=HC)

        wrawp = top.enter_context(tc.tile_pool(name="wraw", bufs=4))

        def w_raw_load(w_dram, ncol, wname):
            rows = w_dram.shape[0]
            nb = rows // 128
            wr = wrawp.tile([128, nb * ncol], F16, tag="wr",
                            name=f"wr_{wname}")
            nc.sync.dma_start(
                out=wr[:].rearrange("p (b c) -> p b c", b=nb),
                in_=w_dram.rearrange("(b p) c -> p b c", p=128))
            return wr

        def w_transpose_group(wr, wt, ncol, nb, wname, g):
            ps = ps_m.tile([128, 512], F16, tag="misc",
                           name=f"wtp_{wname}_{g}")
            for s4 in range(4):
                idx = 4 * g + s4
                kc, bb = divmod(idx, nb)
                nc.tensor.transpose(
                    ps[:, 128 * s4:128 * (s4 + 1)],
                    wr[:, ncol * bb + 128 * kc:ncol * bb + 128 * (kc + 1)],
                    ident[:])
            nc.vector.tensor_copy(wt[:, 512 * g:512 * (g + 1)], ps[:])

        def w_transpose(wr, wt, ncol, nb, wname):
            # wt [128, (ncol/128) * rows]: block kc at cols rows*kc
            npc = ncol // 128
            for g in range(nb * npc // 4):
                w_transpose_group(wr, wt, ncol, nb, wname, g)

        def xT_load(x_dram, xt):
            for kc in range(PD):
                nc.sync.dma_start_transpose(
                    out=xt[:, npad * kc:npad * (kc + 1)],
                    in_=x_dram[:, 128 * kc:128 * (kc + 1)])

        # combined ctx psum group fits one 2KB bank only in the 1152 case
        PIPE = 8 * GW * 2 <= 2048 * 2  # 2*GW f32 <= one bank

        ps_s = top.enter_context(tc.tile_pool(name="ps_s", bufs=2,
                                              space="PSUM"))
        ps_c = top.enter_context(tc.tile_pool(name="ps_c", bufs=2,
                                              space="PSUM"))
        ps_m = top.enter_context(tc.tile_pool(name="ps_m", bufs=2,
                                              space="PSUM"))
        zpool = top.enter_context(
            tc.tile_pool(name="z", bufs=(2 * NKT if PIPE else 4)))
        cpool = top.enter_context(tc.tile_pool(name="cst", bufs=3))
        outsb = top.enter_context(tc.tile_pool(name="osb", bufs=4))
        smalls = top.enter_context(tc.tile_pool(name="sm", bufs=4))

        def proj_T(xt, wt, b_sb, out_sb, m, j):
            ps = ps_m.tile([128, QC], F32, tag="misc", name=f"pj_{m}_{j}")
            for kc in range(PD):
                nc.tensor.matmul(
                    ps[:],
                    lhsT=wt[:, DH * kc + 128 * m:DH * kc + 128 * (m + 1)],
                    rhs=xt[:, npad * kc + QC * j:npad * kc + QC * (j + 1)],
                    start=(kc == 0), stop=(kc == PD - 1))
            nc.vector.tensor_scalar_add(
                out_sb[:, npad * m + QC * j:npad * m + QC * (j + 1)],
                ps[:], b_sb[:, m:m + 1])

        def proj_v(i):
            ps = ps_m.tile([128, QC], F32, tag="misc", name=f"pv_{i}")
            for kc in range(PD):
                nc.tensor.matmul(
                    ps[:, 0:DH],
                    lhsT=xvT[:, npad * kc + 128 * i:
                             npad * kc + 128 * (i + 1)],
                    rhs=wvt[:, DH * kc:DH * (kc + 1)],
                    start=(kc == 0), stop=(kc == PD - 1))
            dst = vag[:, i, :, 0:DK]
            src = ps[:, 0:DH].rearrange("p (h c) -> p h c", h=HC)
            bvr = bv_rep[:].rearrange("p (h c) -> p h c", h=HC)
            nc.vector.tensor_add(dst, src, bvr)
            nc.gpsimd.tensor_scalar_mul(
                dst, dst, vones[:, HC * i:HC * i + 1])

        # ---- DMA order: all 4 weights raw (one DMA each), then xv, xk,
        # xq transposed. PE fills the lead with weight transposes + v/k
        # projections; m=1/j>0 q-proj chunks deferred into stage-C slack.
        wv_r = w_raw_load(wv_d, D, "wv")
        wk_r = w_raw_load(wk_d, D, "wk")
        wq_r = w_raw_load(wq_d, D, "wq")
        xT_load(xv_d, xvT)
        xT_load(xk_d, xkT)
        xT_load(xq_d, xqT)
        wo_r = w_raw_load(wo_d, DH, "wo")
        w_transpose(wv_r, wvt, D, MB, "wv")
        w_transpose(wk_r, wkt, D, MB, "wk")
        w_transpose(wq_r, wqt, D, MB, "wq")

        # bv replicated across partitions (rank-1 with ones row)
        psb = ps_m.tile([128, QC], F32, tag="misc", name="psb")
        nc.tensor.matmul(psb[0:128, 0:DH], lhsT=ones_row[:, 0:128],
                         rhs=bv_row16[:], start=True, stop=True)
        nc.vector.tensor_copy(bv_rep[:], psb[0:128, 0:DH])
        # validity columns of v_aug
        nc.vector.tensor_copy(
            vag[:, :, :, DK:DK + 1],
            vones[:].rearrange("p (t h) -> p t h", t=NKT)[:, :, :, None])

        for i in range(NKT):
            proj_v(i)
        for j in range(NJ):
            for m in range(MB):
                proj_T(xkT, wkt, bk_sb, kT, m, j)
        proj_T(xqT, wqt, bq_sb, qT, 0, 0)
        # deferred work: {(block_idx, slot): [closure, ...]}
        deferred = {}

        def defer(bi, i, fn):
            deferred.setdefault((bi, i), []).append(fn)

        if PIPE:
            items = ([("q", 1, 0)] +
                     [("q", mm, jj) for jj in range(1, NJ)
                      for mm in range(MB)])
            slots = [(0, 0), (0, NKT // 2), (1, 0), (1, NKT // 2), (2, 0),
                     (2, NKT // 2), (3, 0), (3, NKT // 2)]
            for (kind, m, j), sl in zip(items, slots):
                defer(sl[0], sl[1],
                      (lambda m=m, j=j: proj_T(xqT, wqt, bq_sb, qT, m, j)))
            npc_wo = DH // 128
            for g in range(PD * npc_wo // 4):
                defer(0, (1, 2, 3, 5)[g],
                      (lambda g=g: w_transpose_group(wo_r, wot, DH, PD,
                                                     "wo", g)))
        else:
            w_transpose(wo_r, wot, DH, PD, "wo")
            for j in range(NJ):
                for m in range(MB):
                    if (m, j) != (0, 0):
                        proj_T(xqT, wqt, bq_sb, qT, m, j)

        def emit_deferred(bi, i):
            for fn in deferred.pop((bi, i), ()):
                fn()

        # ---- attention: software-pipelined over (j, m) blocks ----
        def alloc_ctx(j, m):
            if PIPE:
                tl = ps_c.tile([128, 2 * GW], F32, tag="ctx",
                               name=f"ctx_{j}_{m}")
                return [tl[:, GW * t:GW * (t + 1)] for t in range(2)]
            return [ps_c.tile([128, GW], F32, tag=f"ctx{t}",
                              name=f"ctx_{j}_{m}_{t}") for t in range(2)]

        def emit_scores(j, m, i):
            sps = ps_s.tile([128, 1024], F32, tag="sps")
            for t in range(2):
                nc.tensor.matmul(
                    sps[:, 512 * t:512 * t + QC],
                    lhsT=kT[64 * t:64 * (t + 1),
                            npad * m + 128 * i:npad * m + 128 * (i + 1)],
                    rhs=qT[64 * t:64 * (t + 1),
                           npad * m + QC * j:npad * m + QC * (j + 1)],
                    start=True, stop=True)
            z = zpool.tile([128, 2 * QC], F16, tag="z",
                           name=f"z_{j}_{m}_{i}")
            src = sps[:].rearrange("p (t c) -> p t c", t=2)[:, :, 0:QC]
            nc.scalar.activation(
                z[:].rearrange("p (t c) -> p t c", t=2),
                src, AF.Exp, scale=0.125)
            return z

        def emit_ctx_chain(blk, g):
            # one (t, qs) group accumulated over ALL key tiles сonsecutively:
            # psum first_mm clears has_written for the whole BANK, so groups
            # sharing a bank must never interleave their accumulations.
            j, m, ctx_ps, zs = blk
            t, qs = divmod(g, QSUB)
            for i in range(NKT):
                nc.tensor.matmul(
                    ctx_ps[t][:, VW * qs:VW * qs + VW],
                    lhsT=zs[i][:, QC * t + 128 * qs:
                               QC * t + 128 * (qs + 1)],
                    rhs=vag[:, i, 2 * m + t, :],
                    start=(i == 0), stop=(i == NKT - 1))

        def finish_block(blk):
            j, m, ctx_ps, zs = blk
            cstage = cpool.tile([128, 2 * QSUB * DK], F16, tag="cst",
                                name=f"cs_{j}_{m}")
            for t in range(2):
                rden = smalls.tile([128, QSUB], F32, tag="rden",
                                   name=f"rd_{j}_{m}_{t}")
                nc.vector.reciprocal(
                    rden[:],
                    ctx_ps[t].rearrange(
                        "p (q c) -> p q c", c=VW)[:, :, DK:DK + 1])
                for qs in range(QSUB):
                    g = QSUB * t + qs
                    nc.vector.tensor_scalar_mul(
                        cstage[:, DK * g:DK * (g + 1)],
                        ctx_ps[t][:, VW * qs:VW * qs + DK],
                        rden[:, qs:qs + 1])
            ctxt_ps = ps_m.tile([128, QC], F16, tag="misc",
                                name=f"ct_{j}_{m}")
            for t in range(2):
                for qs in range(QSUB):
                    g = QSUB * t + qs
                    nc.tensor.transpose(
                        ctxt_ps[64 * t:64 * (t + 1),
                                128 * qs:128 * (qs + 1)],
                        cstage[:, DK * g:DK * (g + 1)],
                        ident[:])
            nc.vector.tensor_copy(
                ctx_sb[:, npad * m + QC * j:npad * m + QC * (j + 1)],
                ctxt_ps[:])
            if m == MB - 1:
                if j == NJ - 1 or not PIPE:
                    emit_outproj(j, tail=(j == NJ - 1))
                else:
                    for dd in range(PD):
                        defer(cur_bi[0] + 1, (0, 1, 2, 3, 5, 6, 7, 8)[dd],
                              (lambda j=j, dd=dd: outproj_dd(j, dd)))

        osb_tiles = {}

        def outproj_dd(j, dd, tail=False):
            if j not in osb_tiles:
                osb_tiles[j] = outsb.tile([128, PD * QC], F16, tag="osb",
                                          name=f"ob_{j}")
            osb = osb_tiles[j]
            ps = ps_m.tile([128, QC], F32, tag="misc",
                           name=f"op_{j}_{dd}")
            for kc in range(MB):
                nc.tensor.matmul(
                    ps[:],
                    lhsT=wot[:, D * kc + 128 * dd:
                             D * kc + 128 * (dd + 1)],
                    rhs=ctx_sb[:, npad * kc + QC * j:
                               npad * kc + QC * (j + 1)],
                    start=(kc == 0), stop=(kc == MB - 1))
            if tail and dd % 2 == 1:
                nc.scalar.copy(osb[:, QC * dd:QC * (dd + 1)], ps[:])
            else:
                nc.vector.tensor_copy(osb[:, QC * dd:QC * (dd + 1)], ps[:])
            if dd == PD - 1:
                nc.sync.dma_start(
                    out=outT_d.rearrange("(dd p) q -> p dd q", p=128)[
                        :, :, QC * j:QC * (j + 1)],
                    in_=osb[:].rearrange("p (dd q) -> p dd q", dd=PD))

        def emit_outproj(j, tail=False):
            for dd in range(PD):
                outproj_dd(j, dd, tail=tail)

        prev = None
        cur_bi = [0]
        for bi, (j, m) in enumerate([(j, m) for j in range(NJ)
                                     for m in range(MB)]):
            cur_bi[0] = bi
            ctx_ps = alloc_ctx(j, m)
            zs = []
            if PIPE:
                last = (j, m) == (NJ - 1, MB - 1)
                blk = (j, m, ctx_ps, zs)
                for i in range(NKT):
                    zs.append(emit_scores(j, m, i))
                    emit_deferred(bi, i)
                    if prev is not None and i < 2 * QSUB:
                        emit_ctx_chain(prev, i)
                if prev is not None:
                    finish_block(prev)
                if last:
                    for g in range(2 * QSUB):
                        emit_ctx_chain(blk, g)
                    finish_block(blk)
                    prev = None
                else:
                    prev = blk
            else:
                blk = (j, m, ctx_ps, zs)
                for i in range(NKT):
                    zs.append(emit_scores(j, m, i))
                for g in range(2 * QSUB):
                    emit_ctx_chain(blk, g)
                finish_block(blk)


        for key in sorted(deferred):
            for fn in deferred.pop(key):
                fn()

    nc.compile()
    return nc


def _get_nc(npad=NKP):
    key = ("nc", npad)
    if key not in _cache:
        _cache[key] = _build_nc(npad)
    return _cache[key]


def _shard_inputs(npad, query, key, value, mask, wq, bq, wk, bk, wv, bv,
                  wo, bo):
    f16 = np.float16
    f32 = np.float32
    in_maps = []
    NKT = npad // 128
    for c in range(NCORES):
        b, r = c // 4, c % 4
        rows = slice(DH * r, DH * (r + 1))
        maskb = np.ascontiguousarray(mask[b, 0]).astype(np.int64)
        idx = np.flatnonzero(maskb)
        idx_pad = np.zeros(npad, np.int64)
        idx_pad[:idx.size] = idx
        valid = np.zeros(npad, f32)
        valid[:idx.size] = 1.0
        vones = np.repeat(valid.reshape(-1, 128).T[:, :, None], HC,
                          axis=2).reshape(128, -1)
        cstp = np.concatenate(
            [vones,
             bq[rows].reshape(MB, 128).T,
             bk[rows].reshape(MB, 128).T], axis=1).astype(f32)
        in_maps.append({
            "xq": np.ascontiguousarray(query[b][idx_pad], f16),
            "xk": np.ascontiguousarray(key[b][idx_pad], f16),
            "xv": np.ascontiguousarray(value[b][idx_pad], f16),
            "wq": np.ascontiguousarray(wq[rows, :], f16),
            "wk": np.ascontiguousarray(wk[rows, :], f16),
            "wv": np.ascontiguousarray(wv[rows, :], f16),
            "wo": np.ascontiguousarray(wo[:, rows], f16),
            "bv": np.ascontiguousarray(bv[rows], f32),
            "cst": np.ascontiguousarray(cstp, f32),
        })
    return in_maps


def kernel(query, key, value, mask, wq, bq, wk, bk, wv, bv, wo, bo,
           _return_bench=False):
    query = np.asarray(query)
    key = np.asarray(key)
    value = np.asarray(value)
    mask = np.asarray(mask)
    wq, bq = np.asarray(wq), np.asarray(bq)
    wk, bk = np.asarray(wk), np.asarray(bk)
    wv, bv = np.asarray(wv), np.asarray(bv)
    wo, bo = np.asarray(wo, np.float32), np.asarray(bo, np.float32)

    nk_max = int(mask.reshape(B, -1).sum(1).max())
    npad = NKP if nk_max <= NKP else S
    nc = _get_nc(npad)
    in_maps = _shard_inputs(npad, query, key, value, mask, wq, bq, wk, bk,
                            wv, bv, wo, bo)
    res = run_bass_kernel_spmd(nc, in_maps, list(range(NCORES)))

    out = np.empty((B, S, D), np.float32)
    for b in range(B):
        acc = res.results[4 * b]["outT"].astype(np.float32)
        for r in range(1, 4):
            acc += res.results[4 * b + r]["outT"]
        maskb = mask[b, 0].astype(bool)
        idx = np.flatnonzero(maskb)
        out_b = np.empty((S, D), np.float32)
        out_b[idx] = acc.T[:idx.size] + bo
        # masked queries: softmax over all-NEG scores is uniform over ALL
        # keys -> ctx = mean(v); constant row per batch
        vmean = np.asarray(value[b], np.float32).mean(0) @ \
            np.asarray(wv, np.float32).T + np.asarray(bv, np.float32)
        vout = vmean @ wo.T + bo
        out_b[~maskb] = vout
        out[b] = out_b
    if _return_bench:
        return out, res
    return out
